# revision 40
# baseline (speedup 1.0000x reference)
"""Trainium2 Bass kernel for additive (Bahdanau-style) attention scoring.

Computes, for hidden [B,H], encoder_outputs [B,S,H], W_attn [2H,H], b_attn [H], v [H]:
    energy    = tanh(hidden @ W1 + enc @ W2 + b_attn)   (per (b,s) row)
    attention = softmax_S(energy @ v)                   -> [B, S]

Sharding: data-parallel over batch across 8 NeuronCores (2 batches/core);
weights replicated.  Per-core compute is a 4096x1024x1024 GEMM + tanh +
v-dot + softmax, laid out as zT tiles [k=128 partitions, r free] so the
tanh bias is a per-partition AP on the scalar engine and the v-dot is a
PE matvec contraction over k.  enc is cast f32->fp16 during the HBM load
(SWDGE) and transposed on-chip with PE identity-matmul transposes whose
PSUM results DVE-copies back to SBUF (cheapest path measured: the xbar
DMA-transpose costs ~1.2us of sequencer time per 128x128 tile, and a
DRAM fp16 staging round-trip starves the HWDGE rings while SWDGE runs).
"""

import sys
import types

import numpy as np

B, S, H = 16, 2048, 1024
N_CORES = 8
B_LOC = B // N_CORES  # 2 batches per core
HC = H // 128         # 8 contraction chunks
KC = H // 128         # 8 output-feature chunks
RB = 512              # rows (s positions) per block
NRB = S // RB         # 4 r-blocks per batch


def _ensure_axon_hooks():
    """Register the NTFF profile hook if the image's antenv lacks it.

    Harmless when tracing is never requested; required for trace=True.
    """
    try:
        import antenv.axon_hooks  # noqa: F401
        return
    except ImportError:
        pass
    try:
        import antenv
        from trn_agent_boot.trn_boot import _ntff_profile_via_ctypes
    except ImportError:
        return
    mod = types.ModuleType("antenv.axon_hooks")
    _hook = [None]
    mod.set_axon_ntff_profile_hook = lambda h: _hook.__setitem__(0, h)
    mod.get_axon_ntff_profile_hook = lambda: _hook[0]
    antenv.axon_hooks = mod
    sys.modules["antenv.axon_hooks"] = mod
    try:
        hook = _ntff_profile_via_ctypes("/opt/axon/libaxon_pjrt.so")
        mod.set_axon_ntff_profile_hook(hook)
    except Exception:
        pass


_ensure_axon_hooks()

import concourse.bass as bass  # noqa: E402,F401
import concourse.mybir as mybir  # noqa: E402
import concourse.tile as tile  # noqa: E402
from concourse import bacc  # noqa: E402
from concourse.bass_utils import run_bass_kernel_spmd  # noqa: E402
from concourse.masks import make_identity  # noqa: E402
from concourse.tile_rust import add_dep_helper  # noqa: E402

f32 = mybir.dt.float32
f16 = mybir.dt.float16
AF = mybir.ActivationFunctionType


def build_kernel():
    nc = bacc.Bacc("TRN2", target_bir_lowering=False, debug=False,
                   num_devices=N_CORES)

    enc = nc.dram_tensor("enc", [B_LOC, S, H], f32, kind="ExternalInput")
    hid = nc.dram_tensor("hid", [B_LOC, H], f32, kind="ExternalInput")
    w_attn = nc.dram_tensor("w_attn", [2 * H, H], f32, kind="ExternalInput")
    b_attn = nc.dram_tensor("b_attn", [H], f32, kind="ExternalInput")
    v = nc.dram_tensor("v", [H], f32, kind="ExternalInput")
    out = nc.dram_tensor("out", [B_LOC, S], f32, kind="ExternalOutput")

    with tile.TileContext(nc) as tc, \
         tc.tile_pool(name="weights", bufs=1) as wpool, \
         tc.tile_pool(name="consts", bufs=1) as cpool, \
         tc.tile_pool(name="nat", bufs=3) as natpool, \
         tc.tile_pool(name="encT", bufs=16) as tpool, \
         tc.tile_pool(name="energy", bufs=9) as epool, \
         tc.tile_pool(name="sm", bufs=1) as smpool, \
         tc.tile_pool(name="psz", bufs=4, space="PSUM") as pszpool, \
         tc.tile_pool(name="psatt", bufs=1, space="PSUM") as psattpool, \
         tc.tile_pool(name="pst", bufs=2, space="PSUM") as pstpool, \
         tc.tile_pool(name="pscb", bufs=1, space="PSUM") as pscbpool:

        # identities first: make_identity runs on the gpsimd queue, and the
        # first PE transpose needs it — ahead of all the Q7 DMA issues
        ident = cpool.tile([128, 128], f16, tag="ident")
        make_identity(nc, ident[:])
        ident2 = cpool.tile([B_LOC, B_LOC], f16, tag="ident2")
        make_identity(nc, ident2[:])

        # --- SWDGE FIFO: first GEMM block's data first, then weights ------
        # One 1MB DMA per r-block (Q7 descriptor generation costs ~0.65us
        # per dma_start, so fewer+bigger issues unblock the prologue)
        nat = {}

        def load_nat(b, rb):
            t = natpool.tile([128, (RB // 128) * H], f16, tag="nat")
            r0 = rb * RB
            nc.gpsimd.dma_start(
                t[:].rearrange("p (j h) -> p j h", h=H),
                enc[b, r0:r0 + RB, :].rearrange("(j p) h -> p j h", p=128))
            nat[(b, rb)] = t

        load_nat(0, 0)

        # W2 by k-columns: GEMM group kc is gated on only its own 0.5 MB
        # column block.  The GEMM consumes columns slower (1.7us/group)
        # than they arrive (~1.05us), so the cascade-critical operands
        # (hidT, battnT, W1col0-1) slot in after W2col3 without starving
        # the GEMM — landing cbias(0) several us earlier.
        w2col = []

        def load_w2col(kc):
            t2 = wpool.tile([128, H], f16, tag=f"w2_{kc}")
            nc.gpsimd.dma_start(
                t2[:].rearrange("p (c k) -> p c k", k=128),
                w_attn[H:2 * H, kc * 128:(kc + 1) * 128].rearrange(
                    "(c p) k -> p c k", p=128))
            w2col.append(t2)

        w1col = []

        def load_w1col(kc):
            t1 = wpool.tile([128, H], f16, tag=f"w1_{kc}")
            nc.gpsimd.dma_start(
                t1[:].rearrange("p (c k) -> p c k", k=128),
                w_attn[0:H, kc * 128:(kc + 1) * 128].rearrange(
                    "(c p) k -> p c k", p=128))
            w1col.append(t1)

        for kc in range(4):
            load_w2col(kc)
        hidT = cpool.tile([128, HC * B_LOC], f16, tag="hidT")
        for b in range(B_LOC):
            nc.gpsimd.dma_start(
                hidT[:].rearrange("p (c b) -> p c b", b=B_LOC)[:, :, b],
                hid[b].rearrange("(c p) -> p c", p=128))
        battnT = cpool.tile([128, KC], f32, tag="battnT")
        nc.gpsimd.dma_start(battnT[:], b_attn.ap().rearrange("(c p) -> p c", p=128))
        load_w1col(0)
        load_w1col(1)
        for kc in range(4, KC):
            load_w2col(kc)
        load_w1col(2)
        load_w1col(3)
        vT = cpool.tile([128, KC], f16, tag="vT")
        nc.gpsimd.dma_start(vT[:], v.ap().rearrange("(c p) -> p c", p=128))
        for kc in range(4, KC):
            load_w1col(kc)

        # remaining enc loads stream behind the small operands
        for b in range(B_LOC):
            for rb in range(NRB):
                if (b, rb) == (0, 0):
                    continue
                load_nat(b, rb)

        # --- cbiasT[k, (kc, b)] = (hidden @ W1 + b_attn) transposed -------
        # per-kc cascade: cbias(kc) is ready as soon as W1col(kc) lands
        hid16 = cpool.tile([B_LOC, H], f16, tag="hid16")
        cbiasT = cpool.tile([128, KC * B_LOC], f32, tag="cbiasT")
        for kc in range(KC):
            psh = pscbpool.tile([B_LOC, 128], f32, tag="pscb")
            for hc in range(HC):
                nc.tensor.matmul(
                    psh[:], hidT[:, hc * B_LOC:(hc + 1) * B_LOC],
                    w1col[kc][:, hc * 128:(hc + 1) * 128],
                    start=(hc == 0), stop=(hc == HC - 1))
            nc.vector.tensor_copy(hid16[:, kc * 128:(kc + 1) * 128], psh[:])
            pstc = pscbpool.tile([128, B_LOC], f16, tag="pscb")
            nc.tensor.transpose(
                pstc[:], hid16[:, kc * 128:(kc + 1) * 128], ident2[:])
            nc.scalar.activation(
                cbiasT[:, kc * B_LOC:(kc + 1) * B_LOC], pstc[:],
                AF.Identity, bias=battnT[:, kc:kc + 1])

        # --- main loop ----------------------------------------------------
        # Phase discipline: all PE transposes of block i+1 are ordered after
        # the last GEMM matmul of block i (same-engine, no semaphore), so the
        # PE alternates pure-transpose and pure-matmul phases.  Interleaving
        # transpose-mode ops into the matmul stream was measured to hold the
        # PE at its cold 1.2 GHz clock (~414 ns vs ~224 ns per N=512 matmul).
        def do_transposes(b, rb, prev_anchor):
            encTs = []
            nt = nat[(b, rb)]
            for hc in range(HC):
                tt = tpool.tile([128, RB], f16, tag="encT")
                pt = pstpool.tile([128, RB], f16, tag="pst")
                for j in range(RB // 128):
                    tr = nc.tensor.transpose(
                        pt[:, j * 128:(j + 1) * 128],
                        nt[:, j * H + hc * 128: j * H + (hc + 1) * 128],
                        ident[:])
                    if prev_anchor is not None:
                        add_dep_helper(prev_anchor.ins, tr.ins,
                                       sync=False, reason="pe phase")
                nc.vector.tensor_copy(tt[:], pt[:])
                encTs.append(tt)
            return encTs

        # Per block i the PE stream is: GEMM(i) x64 -> transposes(i+1) x32
        # -> v-dots(i) x8.  Every instruction's inputs are ready when the
        # in-order PE queue reaches it (the v-dots' tanh deps complete
        # during the transpose phase), so the PE never stalls mid-stream.
        blocks = [(b, rb) for b in range(B_LOC) for rb in range(NRB)]
        logits = {}
        for b in range(B_LOC):
            lg = smpool.tile([1, S], f32, tag=f"logits_{b}")
            logits[b] = lg
        encTs_next = do_transposes(0, 0, None)
        for bi, (b, rb) in enumerate(blocks):
            encTs = encTs_next
            psa = psattpool.tile([1, RB], f32)
            ens = []
            last_g = None
            for kc in range(KC):
                psz = pszpool.tile([128, RB], f32)
                for hc in range(HC):
                    last_g = nc.tensor.matmul(
                        psz[:], w2col[kc][:, hc * 128:(hc + 1) * 128],
                        encTs[hc][:],
                        start=(hc == 0), stop=(hc == HC - 1))
                en = epool.tile([128, RB], f16, tag="energy")
                nc.scalar.activation(
                    en[:], psz[:], AF.Tanh,
                    bias=cbiasT[:, kc * B_LOC + b: kc * B_LOC + b + 1])
                ens.append(en)
            if bi + 1 < len(blocks):
                encTs_next = do_transposes(*blocks[bi + 1], last_g)
            for kc in range(KC):
                nc.tensor.matmul(
                    psa[:], vT[:, kc:kc + 1], ens[kc][:],
                    start=(kc == 0), stop=(kc == KC - 1))
            nc.vector.tensor_copy(
                logits[b][:, rb * RB:(rb + 1) * RB], psa[:])

        for b in range(B_LOC):
            # softmax over S on one partition; logits are O(1) so exp is
            # safe without max-subtraction (matches softmax exactly in math).
            expo2 = smpool.tile([1, S], f32, tag=f"expo2_{b}")
            ssum = smpool.tile([1, 1], f32, tag=f"ssum_{b}")
            nc.scalar.activation(expo2[:], logits[b][:], AF.Exp,
                                 accum_out=ssum[:])
            rec = smpool.tile([1, 1], f32, tag=f"rec_{b}")
            nc.vector.reciprocal(rec[:], ssum[:])
            prob = smpool.tile([1, S], f32, tag=f"prob_{b}")
            nc.scalar.activation(prob[:], expo2[:], AF.Copy, scale=rec[:])
            nc.sync.dma_start(out[b:b + 1, :], prob[:])

    nc.compile()
    return nc


_NC_CACHE = None


def _get_nc():
    global _NC_CACHE
    if _NC_CACHE is None:
        _NC_CACHE = build_kernel()
    return _NC_CACHE


def kernel(hidden, encoder_outputs, W_attn, b_attn, v, _trace=False,
           _tmpdir=None):
    hidden = np.ascontiguousarray(hidden, dtype=np.float32)
    encoder_outputs = np.ascontiguousarray(encoder_outputs, dtype=np.float32)
    W_attn = np.ascontiguousarray(W_attn, dtype=np.float32)
    b_attn = np.ascontiguousarray(b_attn, dtype=np.float32)
    v = np.ascontiguousarray(v, dtype=np.float32)

    nc = _get_nc()
    in_maps = []
    for c in range(N_CORES):
        b0 = c * B_LOC
        in_maps.append({
            "enc": encoder_outputs[b0:b0 + B_LOC],
            "hid": hidden[b0:b0 + B_LOC],
            "w_attn": W_attn,
            "b_attn": b_attn,
            "v": v,
        })
    res = run_bass_kernel_spmd(
        nc, in_maps, core_ids=list(range(N_CORES)),
        trace=_trace, tmpdir=_tmpdir)
    out = np.concatenate([res.results[c]["out"] for c in range(N_CORES)],
                         axis=0).astype(np.float32)
    if _trace:
        kernel.last_exec_time_ns = res.exec_time_ns
        kernel.last_results = res
    return out
